# revision 1
# baseline (speedup 1.0000x reference)
"""Trainium2 Bass kernel for HINGCN-GS (2-metapath, 2-layer GraphSAGE-style GNN).

Strategy (8 NeuronCores, pure data-parallel over the seed batch):
  - B=512 seeds -> 64 seeds per core. feats / edge_emb tables and all
    weights replicated in each core's HBM; index arrays sharded along dim 0.
  - On-device gathers via SWDGE indirect DMA (one descriptor per row).
  - All activations kept feature-major ("transposed", [feat, node]) so every
    matmul is a natural PE op: out[M=dout, N=nodes] = lhsT(W chunk).T @ rhs.
  - Neighbor/edge means become group-sums along the free axis (DVE
    tensor_reduce); the 1/S scaling is folded into W_neigh / W_edge on host.
  - Gathered row-major tiles are transposed on the PE with an identity.
"""

import sys

for _p in ("/opt/trn_rl_repo", "/opt/pypackages"):
    if _p not in sys.path:
        sys.path.insert(0, _p)

import numpy as np

import concourse.bass as bass
import concourse.bacc as bacc
import concourse.mybir as mybir
import concourse.tile as tile
from concourse.masks import make_identity

F32 = mybir.dt.float32
I32 = mybir.dt.int32
AF = mybir.ActivationFunctionType
AX = mybir.AxisListType
ADD = mybir.AluOpType.add

# Problem constants (hardcoded per harness contract)
S = 10
D = 256
E = 64
C = 8
B = 512
NCORES = 8
BC = B // NCORES          # 64 seeds per core
L1 = BC * S               # 640 level-1 nodes per core
L2 = BC * S * S           # 6400 level-2 nodes per core
NT = 100000               # feats rows
NEDGE = 200000            # edge_emb rows
KD = D // 128             # 2 feature chunks of 128
NT1 = 512                 # N-tile split for the 640-wide matmuls
_N_TILES = ((0, 512), (512, 128))


def _build_nc():
    nc = bacc.Bacc(None, target_bir_lowering=False, dynamic_dma_scratch_size=65536)

    # ---- DRAM I/O ----
    feats = nc.dram_tensor("feats", [NT, D], F32, kind="ExternalInput")
    emb = [
        nc.dram_tensor("emb0", [NEDGE, E], F32, kind="ExternalInput"),
        nc.dram_tensor("emb1", [NEDGE, E], F32, kind="ExternalInput"),
    ]
    idx_f0_d = nc.dram_tensor("idx_f0", [BC, 1], I32, kind="ExternalInput")
    idx_f1_d = [nc.dram_tensor(f"idx_f1_{m}", [128, 5], I32, kind="ExternalInput") for m in range(2)]
    idx_e0_d = [nc.dram_tensor(f"idx_e0_{m}", [128, 5], I32, kind="ExternalInput") for m in range(2)]
    idx_f2_d = [nc.dram_tensor(f"idx_f2_{m}", [128, 50], I32, kind="ExternalInput") for m in range(2)]
    idx_e1_d = [nc.dram_tensor(f"idx_e1_{m}", [128, 50], I32, kind="ExternalInput") for m in range(2)]
    ws_d = nc.dram_tensor("ws", [2, 2, 128, KD, D], F32, kind="ExternalInput")
    wn_d = nc.dram_tensor("wn", [2, 2, 128, KD, D], F32, kind="ExternalInput")
    wedge_d = nc.dram_tensor("wedge", [2, 2, E, D], F32, kind="ExternalInput")
    we_d = nc.dram_tensor("we", [2, 128, 5, E], F32, kind="ExternalInput")
    be_d = nc.dram_tensor("be", [2, E, 1], F32, kind="ExternalInput")
    fcw_d = nc.dram_tensor("fcw", [128, KD, C], F32, kind="ExternalInput")
    fcb_d = nc.dram_tensor("fcb", [C, 1], F32, kind="ExternalInput")
    out_d = nc.dram_tensor("out", [BC, C], F32, kind="ExternalOutput")

    with tile.TileContext(nc) as tc:
        with (
            tc.tile_pool(name="singles", bufs=1) as singles,
            tc.tile_pool(name="mp", bufs=2) as mpp,
            tc.tile_pool(name="gchunk", bufs=3) as gch,
            tc.tile_pool(name="ps_t", bufs=2, space="PSUM") as ps_t,
            tc.tile_pool(name="ps_big", bufs=2, space="PSUM") as ps_big,
            tc.tile_pool(name="ps_tiny", bufs=2, space="PSUM") as ps_tiny,
        ):
            ident = singles.tile([128, 128], F32)
            make_identity(nc, ident[:, :])
            ones = singles.tile([128, 1], F32)
            nc.vector.memset(ones[:, :], 1.0)
            ones_row = singles.tile([1, C], F32)
            nc.vector.memset(ones_row[:, :], 1.0)

            # ---- load indices ----
            i_f0 = singles.tile([BC, 1], I32, tag="i_f0")
            nc.sync.dma_start(out=i_f0[:, :], in_=idx_f0_d[:, :])
            i_f1, i_e0, i_f2, i_e1 = [], [], [], []
            for m in range(2):
                t1 = singles.tile([128, 5], I32, tag=f"i_f1_{m}")
                nc.sync.dma_start(out=t1[:, :], in_=idx_f1_d[m][:, :])
                i_f1.append(t1)
                t2 = singles.tile([128, 5], I32, tag=f"i_e0_{m}")
                nc.sync.dma_start(out=t2[:, :], in_=idx_e0_d[m][:, :])
                i_e0.append(t2)
                t3 = singles.tile([128, 50], I32, tag=f"i_f2_{m}")
                nc.sync.dma_start(out=t3[:, :], in_=idx_f2_d[m][:, :])
                i_f2.append(t3)
                t4 = singles.tile([128, 50], I32, tag=f"i_e1_{m}")
                nc.sync.dma_start(out=t4[:, :], in_=idx_e1_d[m][:, :])
                i_e1.append(t4)

            # ---- load weights ----
            ws_t, wn_t, wedge_t = {}, {}, {}
            for m in range(2):
                for l in range(2):
                    w1 = singles.tile([128, KD, D], F32, tag=f"ws_{m}{l}")
                    nc.sync.dma_start(out=w1[:, :, :], in_=ws_d[m, l, :, :, :])
                    ws_t[m, l] = w1
                    w2 = singles.tile([128, KD, D], F32, tag=f"wn_{m}{l}")
                    nc.sync.dma_start(out=w2[:, :, :], in_=wn_d[m, l, :, :, :])
                    wn_t[m, l] = w2
                    w3 = singles.tile([E, D], F32, tag=f"wedge_{m}{l}")
                    nc.sync.dma_start(out=w3[:, :], in_=wedge_d[m, l, :, :])
                    wedge_t[m, l] = w3
            we_t, be_t = [], []
            for m in range(2):
                w4 = singles.tile([128, 5, E], F32, tag=f"we_{m}")
                nc.sync.dma_start(out=w4[:, :, :], in_=we_d[m, :, :, :])
                we_t.append(w4)
                b4 = singles.tile([E, 1], F32, tag=f"be_{m}")
                nc.sync.dma_start(out=b4[:, :], in_=be_d[m, :, :])
                be_t.append(b4)
            fcw_t = singles.tile([128, KD, C], F32)
            nc.sync.dma_start(out=fcw_t[:, :, :], in_=fcw_d[:, :, :])
            fcb_t = singles.tile([C, 1], F32)
            nc.sync.dma_start(out=fcb_t[:, :], in_=fcb_d[:, :])

            # ---- seed feats: gather + transpose (shared by both metapaths) ----
            f0_raw = singles.tile([BC, D], F32)
            nc.gpsimd.indirect_dma_start(
                out=f0_raw[:, :], out_offset=None, in_=feats[:, :],
                in_offset=bass.IndirectOffsetOnAxis(ap=i_f0[:, :], axis=0),
            )
            f0T = singles.tile([128, KD, BC], F32)
            for ck in range(KD):
                ps = ps_t.tile([128, 128], F32, tag="ps_tr")
                nc.tensor.transpose(
                    out=ps[:, :BC], in_=f0_raw[:, ck * 128:(ck + 1) * 128],
                    identity=ident[:BC, :BC],
                )
                nc.scalar.copy(f0T[:, ck, :], ps[:, :BC])

            hsumT = singles.tile([128, KD, BC], F32)

            for m in range(2):
                # ======== gathers ========
                f1_raw = mpp.tile([128, 5, D], F32, tag="f1_raw")
                for t in range(5):
                    nc.gpsimd.indirect_dma_start(
                        out=f1_raw[:, t, :], out_offset=None, in_=feats[:, :],
                        in_offset=bass.IndirectOffsetOnAxis(ap=i_f1[m][:, t:t + 1], axis=0),
                    )
                e0_raw = mpp.tile([128, 5, E], F32, tag="e0_raw")
                for t in range(5):
                    nc.gpsimd.indirect_dma_start(
                        out=e0_raw[:, t, :], out_offset=None, in_=emb[m][:, :],
                        in_offset=bass.IndirectOffsetOnAxis(ap=i_e0[m][:, t:t + 1], axis=0),
                    )

                # ======== f1T / e0T transposes ========
                f1T = mpp.tile([128, KD, L1], F32, tag="f1T")
                for t in range(5):
                    for ck in range(KD):
                        ps = ps_t.tile([128, 128], F32, tag="ps_tr")
                        nc.tensor.transpose(
                            out=ps[:, :], in_=f1_raw[:, t, ck * 128:(ck + 1) * 128],
                            identity=ident[:, :],
                        )
                        nc.vector.tensor_copy(f1T[:, ck, t * 128:(t + 1) * 128], ps[:, :])
                e0T = mpp.tile([E, L1], F32, tag="e0T")
                for t in range(5):
                    ps = ps_t.tile([128, 128], F32, tag="ps_tr")
                    nc.tensor.transpose(
                        out=ps[:E, :], in_=e0_raw[:, t, :], identity=ident[:, :],
                    )
                    nc.scalar.copy(e0T[:, t * 128:(t + 1) * 128], ps[:E, :])

                # group means (sums; 1/S folded into weights on host)
                f1mT = mpp.tile([128, KD, BC], F32, tag="f1mT")
                nc.vector.tensor_reduce(
                    out=f1mT[:, :, :],
                    in_=f1T[:, :, :].rearrange("p c (g s) -> p c g s", s=S),
                    axis=AX.X, op=ADD,
                )
                e0mT = mpp.tile([E, BC], F32, tag="e0mT")
                nc.vector.tensor_reduce(
                    out=e0mT[:, :],
                    in_=e0T[:, :].rearrange("p (g s) -> p g s", s=S),
                    axis=AX.X, op=ADD,
                )

                # ======== level-2 gathers: chunked gather -> reduce -> transpose ====
                f2mT = mpp.tile([128, KD, L1], F32, tag="f2mT")
                e1mT = mpp.tile([E, L1], F32, tag="e1mT")
                for c in range(5):
                    f2c = gch.tile([128, S, D], F32, tag="f2c")
                    for j in range(S):
                        nc.gpsimd.indirect_dma_start(
                            out=f2c[:, j, :], out_offset=None, in_=feats[:, :],
                            in_offset=bass.IndirectOffsetOnAxis(
                                ap=i_f2[m][:, c * S + j:c * S + j + 1], axis=0),
                        )
                    f2s = gch.tile([128, D], F32, tag="f2s")
                    nc.vector.tensor_reduce(
                        out=f2s[:, :],
                        in_=f2c[:, :, :].rearrange("p m d -> p d m"),
                        axis=AX.X, op=ADD,
                    )
                    for ck in range(KD):
                        ps = ps_t.tile([128, 128], F32, tag="ps_tr")
                        nc.tensor.transpose(
                            out=ps[:, :], in_=f2s[:, ck * 128:(ck + 1) * 128],
                            identity=ident[:, :],
                        )
                        nc.vector.tensor_copy(f2mT[:, ck, c * 128:(c + 1) * 128], ps[:, :])

                    e1c = gch.tile([128, S, E], F32, tag="e1c")
                    for j in range(S):
                        nc.gpsimd.indirect_dma_start(
                            out=e1c[:, j, :], out_offset=None, in_=emb[m][:, :],
                            in_offset=bass.IndirectOffsetOnAxis(
                                ap=i_e1[m][:, c * S + j:c * S + j + 1], axis=0),
                        )
                    e1s = gch.tile([128, E], F32, tag="e1s")
                    nc.vector.tensor_reduce(
                        out=e1s[:, :],
                        in_=e1c[:, :, :].rearrange("p m d -> p d m"),
                        axis=AX.X, op=ADD,
                    )
                    ps = ps_t.tile([128, 128], F32, tag="ps_tr")
                    nc.tensor.transpose(
                        out=ps[:E, :], in_=e1s[:, :], identity=ident[:, :],
                    )
                    nc.scalar.copy(e1mT[:, c * 128:(c + 1) * 128], ps[:E, :])

                # ======== layer 0 ========
                # agg k=0: f0' = relu(Ws.f0 + Wn.mean(f1) + Wedge.mean(e0))
                f0pT = mpp.tile([128, KD, BC], F32, tag="f0pT")
                for ck in range(KD):
                    sl = slice(ck * 128, (ck + 1) * 128)
                    ps = ps_tiny.tile([128, BC], F32, tag="ps_agg0")
                    nc.tensor.matmul(ps[:, :], ws_t[m, 0][:, 0, sl], f0T[:, 0, :], start=True, stop=False)
                    nc.tensor.matmul(ps[:, :], ws_t[m, 0][:, 1, sl], f0T[:, 1, :], start=False, stop=False)
                    nc.tensor.matmul(ps[:, :], wn_t[m, 0][:, 0, sl], f1mT[:, 0, :], start=False, stop=False)
                    nc.tensor.matmul(ps[:, :], wn_t[m, 0][:, 1, sl], f1mT[:, 1, :], start=False, stop=False)
                    nc.tensor.matmul(ps[:, :], wedge_t[m, 0][:, sl], e0mT[:, :], start=False, stop=True)
                    nc.scalar.activation(f0pT[:, ck, :], ps[:, :], AF.Relu)

                # agg k=1: f1' = relu(Ws.f1 + Wn.mean(f2) + Wedge.mean(e1))
                f1pT = mpp.tile([128, KD, L1], F32, tag="f1pT")
                for ck in range(KD):
                    sl = slice(ck * 128, (ck + 1) * 128)
                    for (n0, nn) in _N_TILES:
                        nsl = slice(n0, n0 + nn)
                        ps = ps_big.tile([128, NT1], F32, tag="ps_agg1")
                        nc.tensor.matmul(ps[:, :nn], ws_t[m, 0][:, 0, sl], f1T[:, 0, nsl], start=True, stop=False)
                        nc.tensor.matmul(ps[:, :nn], ws_t[m, 0][:, 1, sl], f1T[:, 1, nsl], start=False, stop=False)
                        nc.tensor.matmul(ps[:, :nn], wn_t[m, 0][:, 0, sl], f2mT[:, 0, nsl], start=False, stop=False)
                        nc.tensor.matmul(ps[:, :nn], wn_t[m, 0][:, 1, sl], f2mT[:, 1, nsl], start=False, stop=False)
                        nc.tensor.matmul(ps[:, :nn], wedge_t[m, 0][:, sl], e1mT[:, nsl], start=False, stop=True)
                        nc.scalar.activation(f1pT[:, ck, nsl], ps[:, :nn], AF.Relu)

                # edge update: e0' = tanh(We.[src | dst | e] + be)
                srcrep = mpp.tile([128, KD, L1], F32, tag="srcrep")
                nc.vector.tensor_copy(
                    srcrep[:, :, :].rearrange("p c (g s) -> p c g s", s=S),
                    f0pT[:, :, :].unsqueeze(3).to_broadcast([128, KD, BC, S]),
                )
                e0pT = mpp.tile([E, L1], F32, tag="e0pT")
                for (n0, nn) in _N_TILES:
                    nsl = slice(n0, n0 + nn)
                    ps = ps_big.tile([E, NT1], F32, tag="ps_agg1")
                    nc.tensor.matmul(ps[:, :nn], we_t[m][:, 0, :], srcrep[:, 0, nsl], start=True, stop=False)
                    nc.tensor.matmul(ps[:, :nn], we_t[m][:, 1, :], srcrep[:, 1, nsl], start=False, stop=False)
                    nc.tensor.matmul(ps[:, :nn], we_t[m][:, 2, :], f1pT[:, 0, nsl], start=False, stop=False)
                    nc.tensor.matmul(ps[:, :nn], we_t[m][:, 3, :], f1pT[:, 1, nsl], start=False, stop=False)
                    nc.tensor.matmul(ps[:, :nn], we_t[m][:E, 4, :], e0T[:, nsl], start=False, stop=True)
                    nc.scalar.activation(e0pT[:, nsl], ps[:, :nn], AF.Tanh, bias=be_t[m][:, :])

                # ======== layer 1 ========
                f1pmT = mpp.tile([128, KD, BC], F32, tag="f1pmT")
                nc.vector.tensor_reduce(
                    out=f1pmT[:, :, :],
                    in_=f1pT[:, :, :].rearrange("p c (g s) -> p c g s", s=S),
                    axis=AX.X, op=ADD,
                )
                e0pmT = mpp.tile([E, BC], F32, tag="e0pmT")
                nc.vector.tensor_reduce(
                    out=e0pmT[:, :],
                    in_=e0pT[:, :].rearrange("p (g s) -> p g s", s=S),
                    axis=AX.X, op=ADD,
                )
                for ck in range(KD):
                    sl = slice(ck * 128, (ck + 1) * 128)
                    ps = ps_tiny.tile([128, BC], F32, tag="ps_agg0")
                    nc.tensor.matmul(ps[:, :], ws_t[m, 1][:, 0, sl], f0pT[:, 0, :], start=True, stop=False)
                    nc.tensor.matmul(ps[:, :], ws_t[m, 1][:, 1, sl], f0pT[:, 1, :], start=False, stop=False)
                    nc.tensor.matmul(ps[:, :], wn_t[m, 1][:, 0, sl], f1pmT[:, 0, :], start=False, stop=False)
                    nc.tensor.matmul(ps[:, :], wn_t[m, 1][:, 1, sl], f1pmT[:, 1, :], start=False, stop=False)
                    nc.tensor.matmul(ps[:, :], wedge_t[m, 1][:, sl], e0pmT[:, :], start=False, stop=True)
                    if m == 0:
                        nc.scalar.copy(hsumT[:, ck, :], ps[:, :])
                    else:
                        nc.vector.tensor_add(hsumT[:, ck, :], hsumT[:, ck, :], ps[:, :])

            # ======== head: normalize rows of hsum, then fc ========
            sq = singles.tile([128, KD, BC], F32)
            nc.scalar.activation(sq[:, :, :], hsumT[:, :, :], AF.Square)
            ps_n = ps_tiny.tile([1, BC], F32, tag="ps_agg0")
            nc.tensor.matmul(ps_n[:, :], ones[:, :], sq[:, 0, :], start=True, stop=False)
            nc.tensor.matmul(ps_n[:, :], ones[:, :], sq[:, 1, :], start=False, stop=True)
            n2 = singles.tile([1, BC], F32)
            nc.vector.tensor_copy(n2[:, :], ps_n[:, :])
            nc.vector.tensor_scalar_max(n2[:, :], n2[:, :], 1e-24)
            nrm = singles.tile([1, BC], F32)
            nc.scalar.sqrt(nrm[:, :], n2[:, :])
            rn = singles.tile([1, BC], F32)
            nc.vector.reciprocal(rn[:, :], nrm[:, :])
            # replicate rn across C partitions via PE
            ps_rep = ps_tiny.tile([C, BC], F32, tag="ps_agg0")
            nc.tensor.matmul(ps_rep[:, :], ones_row[:, :], rn[:, :], start=True, stop=True)
            rn_rep = singles.tile([C, BC], F32)
            nc.vector.tensor_copy(rn_rep[:, :], ps_rep[:, :])
            # yT = fc_w.T @ hsum
            ps_y = ps_tiny.tile([C, BC], F32, tag="ps_agg0")
            nc.tensor.matmul(ps_y[:, :], fcw_t[:, 0, :], hsumT[:, 0, :], start=True, stop=False)
            nc.tensor.matmul(ps_y[:, :], fcw_t[:, 1, :], hsumT[:, 1, :], start=False, stop=True)
            y_sc = singles.tile([C, BC], F32)
            nc.vector.tensor_mul(y_sc[:, :], ps_y[:, :], rn_rep[:, :])
            y_b = singles.tile([C, BC], F32)
            nc.scalar.activation(y_b[:, :], y_sc[:, :], AF.Identity, bias=fcb_t[:, :])
            # transpose [C, BC] -> [BC, C] and store
            ps_o = ps_tiny.tile([BC, C], F32, tag="ps_agg0")
            nc.tensor.transpose(out=ps_o[:, :], in_=y_b[:, :], identity=ident[:C, :C])
            out_sb = singles.tile([BC, C], F32)
            nc.vector.tensor_copy(out_sb[:, :], ps_o[:, :])
            nc.sync.dma_start(out=out_d[:, :], in_=out_sb[:, :])

    return nc


_NC_CACHE = {}


def _get_nc():
    if "nc" not in _NC_CACHE:
        nc = _build_nc()
        nc.compile()
        _NC_CACHE["nc"] = nc
    return _NC_CACHE["nc"]


def _prep_in_maps(ids, feats, n00, n01, n10, n11, e00, e01, e10, e11,
                  edge_emb0, edge_emb1, W_self, W_neigh, W_edge, We, be,
                  fc_w, fc_b):
    f32 = np.float32
    feats = np.ascontiguousarray(feats, dtype=f32)
    emb0 = np.ascontiguousarray(edge_emb0, dtype=f32)
    emb1 = np.ascontiguousarray(edge_emb1, dtype=f32)

    ws_host = np.ascontiguousarray(
        np.asarray(W_self, dtype=f32).reshape(2, 2, KD, 128, D).transpose(0, 1, 3, 2, 4))
    wn_host = np.ascontiguousarray(
        (np.asarray(W_neigh, dtype=f32) * (1.0 / S)).reshape(2, 2, KD, 128, D).transpose(0, 1, 3, 2, 4))
    wedge_host = np.ascontiguousarray(np.asarray(W_edge, dtype=f32) * (1.0 / S))
    we_pad = np.zeros((2, 5 * 128, E), dtype=f32)
    we_pad[:, :2 * D + E, :] = np.asarray(We, dtype=f32)[:, 0]
    we_host = np.ascontiguousarray(we_pad.reshape(2, 5, 128, E).transpose(0, 2, 1, 3))
    be_host = np.ascontiguousarray(np.asarray(be, dtype=f32)[:, 0].reshape(2, E, 1))
    fcw_host = np.ascontiguousarray(
        np.asarray(fc_w, dtype=f32).reshape(KD, 128, C).transpose(1, 0, 2))
    fcb_host = np.ascontiguousarray(np.asarray(fc_b, dtype=f32).reshape(C, 1))

    def tile1(x):   # [640] -> [128, 5] row-tile-major
        return np.ascontiguousarray(np.asarray(x, dtype=np.int32).reshape(5, 128).T)

    def tile2(x):   # [6400] -> [128, 50] chunk-major groups, member-contig
        return np.ascontiguousarray(
            np.asarray(x, dtype=np.int32).reshape(5, 128, S).transpose(1, 0, 2).reshape(128, 50))

    neigh1 = (n00, n10)
    neigh2 = (n01, n11)
    eidx1 = (e00, e10)
    eidx2 = (e01, e11)

    in_maps = []
    for c in range(NCORES):
        s0 = slice(c * BC, (c + 1) * BC)
        s1 = slice(c * L1, (c + 1) * L1)
        s2 = slice(c * L2, (c + 1) * L2)
        m = {
            "feats": feats, "emb0": emb0, "emb1": emb1,
            "idx_f0": np.ascontiguousarray(
                np.asarray(ids[s0], dtype=np.int32).reshape(BC, 1)),
            "ws": ws_host, "wn": wn_host, "wedge": wedge_host,
            "we": we_host, "be": be_host, "fcw": fcw_host, "fcb": fcb_host,
        }
        for mp in range(2):
            m[f"idx_f1_{mp}"] = tile1(neigh1[mp][s1])
            m[f"idx_e0_{mp}"] = tile1(eidx1[mp][s1])
            m[f"idx_f2_{mp}"] = tile2(neigh2[mp][s2])
            m[f"idx_e1_{mp}"] = tile2(eidx2[mp][s2])
        in_maps.append(m)
    return in_maps


def kernel(**inputs):
    from concourse.bass_utils import run_bass_kernel_spmd

    nc = _get_nc()
    in_maps = _prep_in_maps(**inputs)
    res = run_bass_kernel_spmd(nc, in_maps, core_ids=list(range(NCORES)))
    out = np.concatenate([r["out"] for r in res.results], axis=0)
    return out.astype(np.float32)

